# revision 1
# baseline (speedup 1.0000x reference)
"""CenterPixelMSE — nn_CenterPixelMSE_11424613007985 — on 8 TRN2 NeuronCores.

loss = mean_b (pred[b, 0, cy_b, cx_b] - target[b])^2
  pred: (512, 1, 256, 256) f32, target: (512,) f32, centers: (512, 2) i32

The loss touches exactly one pixel per batch element, so instead of streaming
the 128 MiB pred tensor, each core gathers its 64 center pixels straight from
HBM with one indirect DMA.

Sharding (pure data parallel over batch, 64 elements per core):
  - pred shard lands in device DRAM untouched; viewed as (64*H*W, 1) so a flat
    element index addresses one pixel.
  - aux input [64, 4] i32 packs, per partition/batch element: cy, cx, the
    constant ramp b*H*W, and the target value's f32 bits.  Packing is pure
    host-side layout (concatenation / bit-view, no arithmetic on data).

Per-core kernel (raw bacc, one instruction per step, waits attached to the
consuming instructions):
  DVE : idx = cy*W + cx            (scalar_tensor_tensor)
  DVE : idx += ramp                (tensor_tensor)
  Pool: g[64,1] = pred[idx]        (indirect SWDGE gather, 64 descriptors)
  DVE : diff = g - target          (tensor_tensor, target via bitcast view)
  PE  : acc[1,1] = diff^T @ diff   (matmul = sum of squared errors)
  DVE : res = acc                  (PSUM -> SBUF)
  SP  : out <- res                 (HWDGE store)

Each core returns its per-shard sum of squared errors; the host all-reduces
the 8 partials and divides by B to form the mean (per the sharding hint).

Notes from hardware iteration:
  - TRN2 allows at most ONE sem wait per instruction; bacc.Bacc.compile()'s
    generate_event_semaphores pass enforces/splits this (plain bass.Bass does
    not run it and multi-wait kernels fail to compile).
  - The indirect-DMA ucode needs one index per SBUF partition ([64,1]); a
    single-partition [1,64] index layout returns garbage on HW (sim accepts it).
  - Park the issuing engine on the DMA completion sem before its end-of-block
    DRAIN: draining a queue with an in-flight DMA delays completion by ~2 us.
"""

import numpy as np

B, H, W = 512, 256, 256
NCORES = 8
BS = B // NCORES  # 64 batch elements per core

_NC_CACHE = {}


def _build_nc():
    import concourse.bass as bass
    import concourse.mybir as mybir
    from concourse import bacc

    nc = bacc.Bacc(
        debug=False,
        enable_asserts=False,
        monotonic_sem_count=0,
        enable_partition_id=False,
    )
    pred = nc.dram_tensor("pred", [BS * H * W, 1], mybir.dt.float32, kind="ExternalInput")
    aux = nc.dram_tensor("aux", [BS, 4], mybir.dt.int32, kind="ExternalInput")
    out = nc.dram_tensor("out", [1, 1], mybir.dt.float32, kind="ExternalOutput")

    ctx = nc.ctx
    A = ctx.enter_context(nc.sbuf_tensor("A", [BS, 4], mybir.dt.int32))
    idx = ctx.enter_context(nc.sbuf_tensor("idx", [BS, 1], mybir.dt.int32))
    g = ctx.enter_context(nc.sbuf_tensor("g", [BS, 1], mybir.dt.float32))
    diff = ctx.enter_context(nc.sbuf_tensor("diff", [BS, 1], mybir.dt.float32))
    res = ctx.enter_context(nc.sbuf_tensor("res", [1, 1], mybir.dt.float32))
    acc = ctx.enter_context(nc.psum_tensor("acc", [1, 1], mybir.dt.float32))

    in_sem = ctx.enter_context(nc.semaphore("in_sem"))
    idx_sem = ctx.enter_context(nc.semaphore("idx_sem"))
    gather_sem = ctx.enter_context(nc.semaphore("gather_sem"))
    diff_sem = ctx.enter_context(nc.semaphore("diff_sem"))
    mm_sem = ctx.enter_context(nc.semaphore("mm_sem"))
    res_sem = ctx.enter_context(nc.semaphore("res_sem"))
    out_sem = ctx.enter_context(nc.semaphore("out_sem"))
    dve_sem = ctx.enter_context(nc.semaphore("dve_sem"))

    cy = A[:, 0:1]
    cx = A[:, 1:2]
    ramp = A[:, 2:3]
    tgt = A[:, 3:4].bitcast(mybir.dt.float32)

    with nc.Block() as block:

        @block.sync
        def _(sync):
            sync.dma_start(out=A[:], in_=aux[:]).then_inc(in_sem, 16)
            sync.dma_start(out=out[:], in_=res[:])._wait_ge(res_sem, 1).then_inc(
                out_sem, 16
            )
            sync.wait_ge(out_sem, 16)

        @block.vector
        def _(vector):
            # idx = cy*W + cx
            vector.scalar_tensor_tensor(
                out=idx[:],
                in0=cy,
                scalar=W,
                in1=cx,
                op0=mybir.AluOpType.mult,
                op1=mybir.AluOpType.add,
            )._wait_ge(in_sem, 16).then_inc(dve_sem, 1)
            # idx += b*H*W (DVE is deeply pipelined: same-engine RAW needs a sem)
            vector.tensor_tensor(
                out=idx[:], in0=idx[:], in1=ramp, op=mybir.AluOpType.add
            )._wait_ge(dve_sem, 1).then_inc(idx_sem, 1)
            vector.tensor_tensor(
                out=diff[:], in0=g[:], in1=tgt, op=mybir.AluOpType.subtract
            )._wait_ge(gather_sem, 16).then_inc(diff_sem, 1)
            vector.tensor_copy(res[:], acc[:])._wait_ge(mm_sem, 1).then_inc(res_sem, 1)

        @block.tensor
        def _(tensor):
            # sum over partitions of diff^2: [1,64]@[64,1]
            tensor.wait_ge(diff_sem, 1)
            tensor.matmul(
                out=acc[:], lhsT=diff[:], rhs=diff[:], start=True, stop=True
            ).then_inc(mm_sem, 1)

        @block.gpsimd
        def _(gpsimd):
            gpsimd.wait_ge(idx_sem, 1)
            gpsimd.indirect_dma_start(
                out=g[:],
                out_offset=None,
                in_=pred[:],
                in_offset=bass.IndirectOffsetOnAxis(ap=idx[:, 0:1], axis=0),
            ).then_inc(gather_sem, 16)
            # Park the engine on the completion sem so the auto-emitted
            # end-of-block queue DRAIN doesn't race the in-flight gather
            # (observed to delay the completion sem by ~1.7us).
            gpsimd.wait_ge(gather_sem, 16)

    nc.compile()
    return nc


def _shard_inputs(pred, target, centers):
    p = np.ascontiguousarray(pred, dtype=np.float32).reshape(NCORES, BS * H * W, 1)
    t = np.ascontiguousarray(target, dtype=np.float32).reshape(NCORES, BS)
    c = np.ascontiguousarray(centers, dtype=np.int32).reshape(NCORES, BS, 2)
    ramp = (np.arange(BS, dtype=np.int64) * (H * W)).astype(np.int32)
    in_maps = []
    for i in range(NCORES):
        aux = np.empty((BS, 4), dtype=np.int32)
        aux[:, 0:2] = c[i]
        aux[:, 2] = ramp
        aux[:, 3] = t[i].view(np.int32)
        in_maps.append({"pred": p[i], "aux": aux})
    return in_maps


def kernel(pred, target, centers, _debug_results=None, **run_kwargs):
    from concourse.bass_utils import run_bass_kernel_spmd

    if "nc" not in _NC_CACHE:
        _NC_CACHE["nc"] = _build_nc()
    nc = _NC_CACHE["nc"]

    in_maps = _shard_inputs(pred, target, centers)
    r = run_bass_kernel_spmd(nc, in_maps, core_ids=list(range(NCORES)), **run_kwargs)
    if _debug_results is not None:
        _debug_results.append(r)
    # Host-side all-reduce of the 8 per-shard sums; divide once to form the mean.
    total = 0.0
    for m in r.results:
        total += float(m["out"].reshape(()))
    return np.asarray(np.float32(total / B))

